# revision 1
# baseline (speedup 1.0000x reference)
"""2x nearest-neighbor upsample of complex (real+imag) NHWC images on 8 trn2 cores.

out[t, b, i, j, c] = x_t[b, i // 2, j // 2, c]   (t = real/imag)

Strategy (data-parallel over batch, 2 images per core):
  - load half an input image row-block into SBUF ([128 rows, 64*64 f32] per half)
  - expand W in SBUF with DVE broadcast copies (each 64-float C-chunk doubled),
    materializing BOTH duplicated output rows in one [128, 16384] tile
  - one store per tile: 3-dim DRAM AP [row i: 128][copy r: 2][8192 contig]
    (walrus caps sync waits per instruction, so fewer DMAs per tile = fewer
    distinct DMA-lane semaphores any instruction must wait on)
HBM traffic per core = 16 MiB read + 64 MiB write (the minimum).
"""

import sys

import numpy as np

if "/opt/trn_rl_repo" not in sys.path:
    sys.path.insert(0, "/opt/trn_rl_repo")

import concourse.bass as bass
import concourse.bass_isa as bass_isa
import concourse.mybir as mybir
import concourse.tile_sem_assignment as _tsa
from concourse.bass_utils import run_bass_kernel_spmd
from concourse.tile import TileContext
from concourse.tile_rust import add_dep_helper

# Partition HWDGE DMA-completion semaphore lanes by issuing engine: SP
# (loads) on lane 0, ACT (stores) on lanes 2-7. Each lane then carries
# DMAs from a single HWDGE FIFO ring (per-lane completion order is
# trivially sound), and a DMA's own-lane predecessor is always one the
# issuing engine has already observed — keeping every DMA at the 1
# sync-wait walrus codegen allows.
_orig_assign_tick = _tsa.TileClockTick._assign_tick


def _assign_tick_lane_split(self, inst):
    if isinstance(inst, _tsa.DMAInst) and not isinstance(
        inst, bass_isa.UserSyncedRemoteDMADescs
    ):
        if inst.engine == mybir.EngineType.Pool:
            self.next_sw_dma_idx = 0
        elif inst.engine == mybir.EngineType.SP:
            self.next_hw_dma_idx = 0
        elif inst.engine == mybir.EngineType.Activation:
            r = getattr(self, "_act_lane_rr", 0)
            self.next_hw_dma_idx = 2 + r
            self._act_lane_rr = (r + 1) % 6
    return _orig_assign_tick(self, inst)


_tsa.TileClockTick._assign_tick = _assign_tick_lane_split

F32 = mybir.dt.float32

B, H, W, C = 16, 128, 128, 64
N_CORES = 8
BPC = B // N_CORES  # images per core

# W-chunk schedule per (tensor, image): halves in steady state; quarters
# for the first image (stores start sooner -> shorter pipeline ramp) and
# the last image (the final store drains sooner -> shorter tail).
_CHUNKS: list[list[tuple[int, int]]] = []
for _t in range(2):
    for _b in range(BPC):
        if _t == 0 and _b == 0:
            _CHUNKS.append([(0, W // 4), (W // 4, W // 4), (W // 2, W // 2)])
        elif _t == 1 and _b == BPC - 1:
            _CHUNKS.append([(0, W // 2), (W // 2, W // 4), (3 * W // 4, W // 4)])
        else:
            _CHUNKS.append([(0, W // 2), (W // 2, W // 2)])
N_ITERS = sum(len(c) for c in _CHUNKS)


def _build() -> bass.Bass:
    nc = bass.Bass("TRN2", debug=False)
    xr = nc.dram_tensor("x_real", [BPC, H, W, C], F32, kind="ExternalInput").ap()
    xi = nc.dram_tensor("x_imag", [BPC, H, W, C], F32, kind="ExternalInput").ap()
    out = nc.dram_tensor(
        "out", [2, BPC, 2 * H, 2 * W, C], F32, kind="ExternalOutput"
    ).ap()
    WH = W // 2  # input W columns per half-tile

    HALF = 2 * WH * C  # expanded half-row length (8192 f32 = 32 KB)

    # walrus codegen allows exactly ONE sync-wait command per engine
    # instruction (multi-wait is only legal on Drain/EventSemaphore).
    # Tile emits a wait only when the issuing engine has not already
    # observed that semaphore tick through an earlier *real* instruction's
    # wait (NoOps don't count). So every instruction below is budgeted to
    # observe at most one fresh tick, using tiny "absorber" instructions
    # (1-element memsets on DVE, 2-element probe copies on ACT, 4-byte
    # writes on SP) to pre-observe everything else.
    #
    # Loads issue from the SP HWDGE ring and stores from the ACT ring so
    # load prefetch is never blocked behind a store's data wait; each
    # store half fires as soon as its own DVE copy finishes.
    with TileContext(nc) as tc:
        with (
            tc.tile_pool(name="pin", bufs=2) as pin,
            tc.tile_pool(name="pinit", bufs=2) as pinit,
            tc.tile_pool(name="pout", bufs=2) as pout,
            tc.tile_pool(name="pdummy", bufs=1) as pdummy,
        ):
            dummy = pdummy.tile([H, 4 * N_ITERS], F32, name="dummy")
            vdummy = pdummy.tile([H, 3 * N_ITERS], F32, name="vdummy")
            pooldummy = pdummy.tile([1, N_ITERS], F32, name="pooldummy")
            spdummy = pdummy.tile([1, 16], F32, name="spdummy")
            last_pabs = None
            st_los = []
            st_his = []
            aabs1s = []
            dmas = []
            cps_all = []
            k = 0
            for t, x in enumerate((xr, xi)):
                for b in range(BPC):
                    # partition i holds input row i, feeding output rows 2i, 2i+1
                    ov = out[t, b].rearrange("(i r) w c -> i r (w c)", r=2)
                    for w0, wlen in _CHUNKS[t * BPC + b]:
                        EXP = 2 * wlen * C  # expanded chunk per output row copy
                        if k < 2:
                            # dedicated, never-recycled tiles: the first two
                            # loads carry no WAW/WAR deps at all
                            tin = pinit.tile(
                                [H, (W // 4) * C], F32, name="tin_init"
                            )
                        else:
                            tin = pin.tile([H, WH * C], F32, name="tin")
                        # Pool-side absorber (gpsimd memset = a real engine
                        # instruction): observe DVE at the newest finished
                        # copy so the load's WAR on its recycled tin slot
                        # (and the slot-release bundle, which lands later
                        # on the DVE timeline than the slot's accessors)
                        # needs no fresh DVE wait.
                        if k >= 2:
                            pabs = nc.gpsimd.memset(pooldummy[:1, k : k + 1], 0.0)
                            add_dep_helper(
                                pabs.ins, cps_all[-1].ins, sync=True,
                                reason="Pool observes DVE for load WAR",
                            )
                            last_pabs = pabs
                            ld = nc.gpsimd.dma_start(
                                out=tin[:, : wlen * C],
                                in_=x[b, :, w0 : w0 + wlen, :],
                            )
                            add_dep_helper(
                                ld.ins, pabs.ins, sync=False,
                                reason="absorber runs before load",
                            )
                        else:
                            # first two chunks: fresh slots, no WAR -> use
                            # the otherwise-idle SP HWDGE ring (faster
                            # first byte than the SWDGE Q7 path)
                            ld = nc.sync.dma_start(
                                out=tin[:, : wlen * C],
                                in_=x[b, :, w0 : w0 + wlen, :],
                            )
                        tout = pout.tile([H, 2 * HALF], F32, name="tout")
                        # DVE-side absorbers: per-iter distinct scratch
                        # cells (no WAW chains), not touching tout (the
                        # slot-release bundle must land on cp0, after the
                        # absorbers already observed all of it).
                        vabs1 = nc.vector.memset(vdummy[:1, 3 * k : 3 * k + 1], 0.0)
                        vabs2 = nc.vector.memset(
                            vdummy[:1, 3 * k + 1 : 3 * k + 2], 0.0
                        )
                        vabs3 = nc.vector.memset(
                            vdummy[:1, 3 * k + 2 : 3 * k + 3], 0.0
                        )
                        if k >= 2:
                            add_dep_helper(
                                vabs1.ins, st_los[k - 2].ins, sync=True,
                                reason="absorb tout slot WAR (store-lo lane)",
                            )
                            add_dep_helper(
                                vabs2.ins, st_his[k - 2].ins, sync=True,
                                reason="absorb tout slot WAR (store-hi lane)",
                            )
                        if k >= 1:
                            add_dep_helper(
                                vabs3.ins, aabs1s[k - 1].ins, sync=True,
                                reason="absorb probe WAR (ACT sem)",
                            )
                        src = (
                            tin[:, : wlen * C]
                            .rearrange("p (w c) -> p w c", c=C)
                            .unsqueeze(2)
                            .broadcast_to([H, wlen, 2, C])
                        )
                        cps = []
                        for r in range(2):
                            dst = tout[:, r * EXP : (r + 1) * EXP].rearrange(
                                "p (w s c) -> p w s c", s=2, c=C
                            )
                            cp = nc.vector.tensor_copy(out=dst, in_=src)
                            for vb in (vabs1, vabs2, vabs3):
                                add_dep_helper(
                                    cp.ins, vb.ins, sync=False,
                                    reason="absorbers run before copies",
                                )
                            cps.append(cp)
                        add_dep_helper(
                            cps[1].ins, cps[0].ins, sync=True,
                            reason="DVE self-sem watermark",
                        )
                        # Each store half fires right after its own copy;
                        # a 2-element ACT probe of that copy's region
                        # absorbs the DVE data wait first.
                        o0 = 2 * w0 * C
                        aabs0 = nc.scalar.copy(
                            out=dummy[:1, 4 * k : 4 * k + 2],
                            in_=tout[:1, 0:2],
                        )
                        st_lo = nc.scalar.dma_start(
                            out=ov[:, 0, o0 : o0 + EXP],
                            in_=tout[:, :EXP],
                        )
                        add_dep_helper(
                            st_lo.ins, aabs0.ins, sync=False,
                            reason="probe runs before store",
                        )
                        aabs1 = nc.scalar.copy(
                            out=dummy[:1, 4 * k + 2 : 4 * k + 4],
                            in_=tout[:1, EXP : EXP + 2],
                        )
                        st_hi = nc.scalar.dma_start(
                            out=ov[:, 1, o0 : o0 + EXP],
                            in_=tout[:, EXP : 2 * EXP],
                        )
                        add_dep_helper(
                            st_hi.ins, aabs1.ins, sync=False,
                            reason="probe runs before store",
                        )
                        st_los.append(st_lo)
                        st_his.append(st_hi)
                        aabs1s.append(aabs1)
                        dmas.extend([ld, st_lo, st_hi])
                        cps_all.extend(cps)
                        k += 1
            # Kernel-tail absorbers: Tile's final SP drain waits on every
            # outstanding proc (DVE + ACT + 8 DMA lanes = 10 waits), but a
            # multi-wait drain lowers to a 1-wait NOP struct when cheap.
            # Pre-observe each proc with one 4-byte SP write per tick.
            # dmas[3] = the second SP-issued head load: its wait covers the
            # DMAHW0 lane both head loads completed on
            tail_deps = dmas[-8:] + [aabs1s[-1], cps_all[-1], last_pabs, dmas[3]]
            for j, dep in enumerate(tail_deps):
                wr = nc.sync.write(spdummy[:1, j : j + 1], b"\x00\x00\x00\x00")
                add_dep_helper(
                    wr.ins, dep.ins, sync=True,
                    reason="pre-observe outstanding procs for tail drain",
                )
    return nc


_NC_CACHE: bass.Bass | None = None


def _get_nc() -> bass.Bass:
    global _NC_CACHE
    if _NC_CACHE is None:
        _NC_CACHE = _build()
    return _NC_CACHE


def _run(x_real: np.ndarray, x_imag: np.ndarray, **spmd_kwargs):
    x_real = np.ascontiguousarray(np.asarray(x_real, dtype=np.float32))
    x_imag = np.ascontiguousarray(np.asarray(x_imag, dtype=np.float32))
    assert x_real.shape == (B, H, W, C), x_real.shape
    assert x_imag.shape == (B, H, W, C), x_imag.shape
    in_maps = [
        {
            "x_real": x_real[c * BPC : (c + 1) * BPC],
            "x_imag": x_imag[c * BPC : (c + 1) * BPC],
        }
        for c in range(N_CORES)
    ]
    res = run_bass_kernel_spmd(
        _get_nc(), in_maps, core_ids=list(range(N_CORES)), **spmd_kwargs
    )
    full = np.concatenate([r["out"] for r in res.results], axis=1)
    return full, res


def kernel(x_real: np.ndarray, x_imag: np.ndarray) -> np.ndarray:
    full, _ = _run(x_real, x_imag)
    return full



# revision 10
# speedup vs baseline: 1.0269x; 1.0269x over previous
"""2x nearest-neighbor upsample of complex (real+imag) NHWC images on 8 trn2 cores.

out[t, b, i, j, c] = x_t[b, i // 2, j // 2, c]   (t = real/imag)

Strategy (data-parallel over batch, 2 images per core):
  - load a W-chunk of all 128 input rows into SBUF (partition i = row i)
  - ONE DVE broadcast copy expands W in SBUF (each 64-float C-block doubled)
  - output rows 2i and 2i+1 are identical, so BOTH row-copy stores read the
    SAME expanded tile -> one copy feeds two stores
  - loads prefetch LD_BUFS=4 chunks ahead (gated on cp(k-3), not cp(k-1) as
    before -> the next chunk's store stream never waits on a load landing)
  - stores stream on the ACT HWDGE ring, round-robined over 6 completion
    lanes; DVE absorbers observe the (k-3) store pair to free the tout slot
HBM traffic per core = 16 MiB read + 64 MiB write (the minimum).
"""

import sys

import numpy as np

if "/opt/trn_rl_repo" not in sys.path:
    sys.path.insert(0, "/opt/trn_rl_repo")

import concourse.bass as bass
import concourse.bass_isa as bass_isa
import concourse.mybir as mybir
import concourse.tile_sem_assignment as _tsa
from concourse.bass_utils import run_bass_kernel_spmd
from concourse.tile import TileContext
from concourse.tile_rust import add_dep_helper

# Partition HWDGE DMA-completion semaphore lanes by issuing engine: SP
# (head loads) alternating lanes 0/1, ACT (stores) on lanes 2-7 round
# robin. Each lane then carries DMAs from a single HWDGE FIFO ring, and a
# DMA's own-lane predecessor is 3 chunks old -- its completion wait (the
# one sync-wait walrus codegen allows per DMA) is satisfied on arrival.
_orig_assign_tick = _tsa.TileClockTick._assign_tick


def _assign_tick_lane_split(self, inst):
    if isinstance(inst, _tsa.DMAInst) and not isinstance(
        inst, bass_isa.UserSyncedRemoteDMADescs
    ):
        if inst.engine == mybir.EngineType.Pool:
            self.next_sw_dma_idx = 0
        elif inst.engine == mybir.EngineType.SP:
            n = getattr(self, "_sp_lane_rr", 0)
            self.next_hw_dma_idx = n
            self._sp_lane_rr = (n + 1) % 2
        elif inst.engine == mybir.EngineType.Activation:
            r = getattr(self, "_act_lane_rr", 0)
            self.next_hw_dma_idx = 2 + r
            self._act_lane_rr = (r + 1) % 6
    return _orig_assign_tick(self, inst)


_tsa.TileClockTick._assign_tick = _assign_tick_lane_split

F32 = mybir.dt.float32

B, H, W, C = 16, 128, 128, 64
N_CORES = 8
BPC = B // N_CORES  # images per core

LD_BUFS = 4  # load prefetch depth (tin slots)
CP_BUFS = 2  # expanded-tile slots

# W-chunk schedule per (tensor, image): halves in steady state; eighths/
# quarters at the very start (first store enqueues sooner -> short ramp)
# and at the very end (small final store -> short drain tail).
E, Q, HF = W // 8, W // 4, W // 2
_CHUNKS: list[list[tuple[int, int]]] = []
for _t in range(2):
    for _b in range(BPC):
        if _t == 0 and _b == 0:
            _CHUNKS.append([(0, E), (E, E), (Q, Q), (HF, HF)])
        elif _t == 1 and _b == BPC - 1:
            _CHUNKS.append([(0, HF), (HF, Q), (3 * Q, E), (3 * Q + E, E)])
        else:
            _CHUNKS.append([(0, HF), (HF, HF)])
_FLAT = [
    (t, b, w0, wlen)
    for t in range(2)
    for b in range(BPC)
    for (w0, wlen) in _CHUNKS[t * BPC + b]
]
N_ITERS = len(_FLAT)


def _build() -> bass.Bass:
    nc = bass.Bass("TRN2", debug=False)
    xr = nc.dram_tensor("x_real", [BPC, H, W, C], F32, kind="ExternalInput").ap()
    xi = nc.dram_tensor("x_imag", [BPC, H, W, C], F32, kind="ExternalInput").ap()
    out = nc.dram_tensor(
        "out", [2, BPC, 2 * H, 2 * W, C], F32, kind="ExternalOutput"
    ).ap()
    xs = (xr, xi)
    EXPMAX = 2 * HF * C  # largest expanded chunk (8192 f32 = 32 KB/partition)

    # walrus codegen allows exactly ONE sync-wait command per engine
    # instruction (multi-wait is only legal on Drain/EventSemaphore). Tile
    # emits a wait only when the issuing engine has not already observed
    # that semaphore tick through an earlier *real* instruction's wait
    # (InstWrite/NoOps don't count). Every instruction below is budgeted to
    # observe at most one fresh tick, using tiny absorber instructions
    # (1-element memsets on DVE, 2-element probe copies on ACT, gpsimd
    # memsets on Pool) to pre-observe everything else; a DMA's remaining
    # single wait is then its own-lane predecessor completion.
    with TileContext(nc) as tc:
        with (
            tc.tile_pool(name="pin", bufs=LD_BUFS) as pin,
            tc.tile_pool(name="pinit", bufs=LD_BUFS) as pinit,
            tc.tile_pool(name="pout", bufs=CP_BUFS) as pout,
            tc.tile_pool(name="pdummy", bufs=1) as pdummy,
        ):
            dummy = pdummy.tile([H, 2 * N_ITERS], F32, name="dummy")
            vdummy = pdummy.tile([H, 4 * N_ITERS], F32, name="vdummy")
            pooldummy = pdummy.tile([1, N_ITERS], F32, name="pooldummy")
            spdummy = pdummy.tile([1, 16], F32, name="spdummy")
            lds = []
            cps = []
            st_los = []
            st_his = []
            aabs_all = []
            last_pabs = None
            for k, (t, b, w0, wlen) in enumerate(_FLAT):
                # ---- load ----
                if k < LD_BUFS:
                    # dedicated, never-recycled tiles: the first LD_BUFS
                    # loads carry no WAR at all; SP HWDGE lanes 0/1 (fast
                    # first byte, two lanes so ramp loads overlap)
                    tin = pinit.tile([H, wlen * C], F32, name="tin_init")
                    ld = nc.sync.dma_start(
                        out=tin[:, : wlen * C],
                        in_=xs[t][b, :, w0 : w0 + wlen, :],
                    )
                else:
                    tin = pin.tile([H, HF * C], F32, name="tin")
                    # Pool absorber (gpsimd memset = a real engine
                    # instruction): observe the DVE tick covering the tin
                    # slot's release (the pool release bundle lands on the
                    # DVE timeline with the NEXT chunk's copy, i.e.
                    # cp(k-LD_BUFS+1)) so the load's single sync wait is
                    # its own-lane predecessor.
                    pabs = nc.gpsimd.memset(pooldummy[:1, k : k + 1], 0.0)
                    add_dep_helper(
                        pabs.ins, cps[k - LD_BUFS + 1].ins, sync=True,
                        reason="Pool observes DVE for load WAR",
                    )
                    last_pabs = pabs
                    ld = nc.gpsimd.dma_start(
                        out=tin[:, : wlen * C],
                        in_=xs[t][b, :, w0 : w0 + wlen, :],
                    )
                    add_dep_helper(
                        ld.ins, pabs.ins, sync=False,
                        reason="absorber runs before load",
                    )
                lds.append(ld)

                # ---- expand (one copy; both output rows read it) ----
                # DVE absorbers: per-iter distinct scratch cells. vabs1/2
                # observe the (k-CP_BUFS) store pair's lane ticks (tout
                # slot WAR); vabs3 observes the newest ACT probe (probe
                # WAR on the recycled tout slot).
                vabs1 = nc.vector.memset(vdummy[:1, 4 * k : 4 * k + 1], 0.0)
                vabs2 = nc.vector.memset(vdummy[:1, 4 * k + 1 : 4 * k + 2], 0.0)
                vabs3 = nc.vector.memset(vdummy[:1, 4 * k + 2 : 4 * k + 3], 0.0)
                vabs4 = nc.vector.memset(vdummy[:1, 4 * k + 3 : 4 * k + 4], 0.0)
                if k >= 1:
                    # DVE self-sem watermark: the recycled tout slot's
                    # release bundle lands at cp(k-1)'s tick on the DVE
                    # timeline; observe it here so the copy's only fresh
                    # wait is its load's data tick.
                    add_dep_helper(
                        vabs4.ins, cps[k - 1].ins, sync=True,
                        reason="absorb tout slot release (DVE self sem)",
                    )
                if k >= CP_BUFS:
                    add_dep_helper(
                        vabs1.ins, st_los[k - CP_BUFS].ins, sync=True,
                        reason="absorb tout slot WAR (store-lo lane)",
                    )
                    add_dep_helper(
                        vabs2.ins, st_his[k - CP_BUFS].ins, sync=True,
                        reason="absorb tout slot WAR (store-hi lane)",
                    )
                if k >= 1:
                    add_dep_helper(
                        vabs3.ins, aabs_all[k - 1].ins, sync=True,
                        reason="absorb probe WAR (ACT sem)",
                    )
                tout = pout.tile([H, EXPMAX], F32, name="tout")
                EXP = 2 * wlen * C
                src = (
                    tin[:, : wlen * C]
                    .rearrange("p (w c) -> p w c", c=C)
                    .unsqueeze(2)
                    .broadcast_to([H, wlen, 2, C])
                )
                dst = tout[:, :EXP].rearrange("p (w s c) -> p w s c", s=2, c=C)
                cp = nc.vector.tensor_copy(out=dst, in_=src)
                for vb in (vabs1, vabs2, vabs3, vabs4):
                    add_dep_helper(
                        cp.ins, vb.ins, sync=False,
                        reason="absorbers run before copy",
                    )
                cps.append(cp)

                # ---- stores (both rows from the same expanded tile) ----
                # One 2-element ACT probe absorbs the DVE data tick; both
                # stores then carry only their own-lane predecessor wait
                # (3 chunks old -> satisfied on arrival).
                ov = out[t, b].rearrange("(i r) w c -> i r (w c)", r=2)
                o0 = 2 * w0 * C
                aabs = nc.scalar.copy(
                    out=dummy[:1, 2 * k : 2 * k + 2], in_=tout[:1, 0:2]
                )
                aabs_all.append(aabs)
                st_lo = nc.scalar.dma_start(
                    out=ov[:, 0, o0 : o0 + EXP], in_=tout[:, :EXP]
                )
                add_dep_helper(
                    st_lo.ins, aabs.ins, sync=False,
                    reason="probe runs before store",
                )
                st_hi = nc.scalar.dma_start(
                    out=ov[:, 1, o0 : o0 + EXP], in_=tout[:, :EXP]
                )
                add_dep_helper(
                    st_hi.ins, st_lo.ins, sync=False,
                    reason="pair stores issue back to back",
                )
                st_los.append(st_lo)
                st_his.append(st_hi)

            # Kernel-tail absorbers: Tile's final SP drain waits on every
            # outstanding proc, but a multi-wait drain lowers to a 1-wait
            # NOP struct when cheap. Pre-observe each proc with one 4-byte
            # SP write per tick: the newest store on each of the 6 ACT
            # lanes, the last gpsimd load (SW lane), the last SP head load
            # (HW lanes 0/1 via lds[2], lds[3]), the last copy (DVE), the
            # last probe (ACT) and the last pool absorber (Pool).
            tail_deps = []
            for j in range(3):
                tail_deps += [st_los[N_ITERS - 1 - j], st_his[N_ITERS - 1 - j]]
            tail_deps += [lds[-1], lds[2], lds[3], cps[-1], aabs_all[-1]]
            if last_pabs is not None:
                tail_deps.append(last_pabs)
            for j, dep in enumerate(tail_deps):
                wr = nc.sync.write(spdummy[:1, j : j + 1], b"\x00\x00\x00\x00")
                add_dep_helper(
                    wr.ins, dep.ins, sync=True,
                    reason="pre-observe outstanding procs for tail drain",
                )
    return nc


_NC_CACHE: bass.Bass | None = None


def _get_nc() -> bass.Bass:
    global _NC_CACHE
    if _NC_CACHE is None:
        _NC_CACHE = _build()
    return _NC_CACHE


def _run(x_real: np.ndarray, x_imag: np.ndarray, **spmd_kwargs):
    x_real = np.ascontiguousarray(np.asarray(x_real, dtype=np.float32))
    x_imag = np.ascontiguousarray(np.asarray(x_imag, dtype=np.float32))
    assert x_real.shape == (B, H, W, C), x_real.shape
    assert x_imag.shape == (B, H, W, C), x_imag.shape
    in_maps = [
        {
            "x_real": x_real[c * BPC : (c + 1) * BPC],
            "x_imag": x_imag[c * BPC : (c + 1) * BPC],
        }
        for c in range(N_CORES)
    ]
    res = run_bass_kernel_spmd(
        _get_nc(), in_maps, core_ids=list(range(N_CORES)), **spmd_kwargs
    )
    full = np.concatenate([r["out"] for r in res.results], axis=1)
    return full, res


def kernel(x_real: np.ndarray, x_imag: np.ndarray) -> np.ndarray:
    full, _ = _run(x_real, x_imag)
    return full


# revision 11
# speedup vs baseline: 1.0470x; 1.0196x over previous
"""2x nearest-neighbor upsample of complex (real+imag) NHWC images on 8 trn2 cores.

out[t, b, i, j, c] = x_t[b, i // 2, j // 2, c]   (t = real/imag)

Strategy (data-parallel over batch, 2 images per core):
  - load a W-chunk of all 128 input rows into SBUF (partition i = row i)
  - ONE DVE broadcast copy expands W in SBUF (each 64-float C-block doubled)
  - output rows 2i and 2i+1 are identical, so BOTH row-copy stores read the
    SAME expanded tile -> one copy feeds two stores
  - everything DMAs through HWDGE rings only (no SWDGE descriptor-ring SBUF
    traffic, which slows SDMA engines 7/15): the 4 ramp loads on SP lanes
    0/1, then each steady load ld(k) rides the ACT store ring inside chunk
    (k-3)'s stream -- enqueued 3 chunks early, it lands well before its
    copy needs it, and the probe of that chunk has already observed the
    DVE tick its tin-slot WAR needs
HBM traffic per core = 16 MiB read + 64 MiB write (the minimum).
"""

import sys

import numpy as np

if "/opt/trn_rl_repo" not in sys.path:
    sys.path.insert(0, "/opt/trn_rl_repo")

import concourse.bass as bass
import concourse.bass_isa as bass_isa
import concourse.mybir as mybir
import concourse.tile_sem_assignment as _tsa
from concourse.bass_utils import run_bass_kernel_spmd
from concourse.tile import TileContext
from concourse.tile_rust import add_dep_helper

# Partition HWDGE DMA-completion semaphore lanes by issuing engine: SP
# (ramp loads) alternating lanes 0/1, ACT (stores + steady loads) on lanes
# 2-7 round robin. Each lane then carries DMAs from a single HWDGE FIFO
# ring, and a DMA's own-lane predecessor is ~2 chunks old -- its
# completion wait (the one sync-wait walrus codegen allows per DMA) is
# satisfied on arrival.
_orig_assign_tick = _tsa.TileClockTick._assign_tick


def _assign_tick_lane_split(self, inst):
    if isinstance(inst, _tsa.DMAInst) and not isinstance(
        inst, bass_isa.UserSyncedRemoteDMADescs
    ):
        if inst.engine == mybir.EngineType.Pool:
            self.next_sw_dma_idx = 0
        elif inst.engine == mybir.EngineType.SP:
            n = getattr(self, "_sp_lane_rr", 0)
            self.next_hw_dma_idx = n
            self._sp_lane_rr = (n + 1) % 2
        elif inst.engine == mybir.EngineType.Activation:
            r = getattr(self, "_act_lane_rr", 0)
            self.next_hw_dma_idx = 2 + r
            self._act_lane_rr = (r + 1) % 6
    return _orig_assign_tick(self, inst)


_tsa.TileClockTick._assign_tick = _assign_tick_lane_split

F32 = mybir.dt.float32

B, H, W, C = 16, 128, 128, 64
N_CORES = 8
BPC = B // N_CORES  # images per core

LD_BUFS = 4  # load prefetch depth (tin slots)
CP_BUFS = 2  # expanded-tile slots

# W-chunk schedule per (tensor, image): halves in steady state; eighths/
# quarters only at the very start (first store enqueues sooner -> short
# ramp). The tail stays on halves: small tail chunks pay more in serial
# load->copy->store latency than their shorter final drain saves.
E, Q, HF = W // 8, W // 4, W // 2
_CHUNKS: list[list[tuple[int, int]]] = []
for _t in range(2):
    for _b in range(BPC):
        if _t == 0 and _b == 0:
            _CHUNKS.append([(0, E), (E, E), (Q, Q), (HF, HF)])
        else:
            _CHUNKS.append([(0, HF), (HF, HF)])
_FLAT = [
    (t, b, w0, wlen)
    for t in range(2)
    for b in range(BPC)
    for (w0, wlen) in _CHUNKS[t * BPC + b]
]
N_ITERS = len(_FLAT)


def _build() -> bass.Bass:
    nc = bass.Bass("TRN2", debug=False)
    xr = nc.dram_tensor("x_real", [BPC, H, W, C], F32, kind="ExternalInput").ap()
    xi = nc.dram_tensor("x_imag", [BPC, H, W, C], F32, kind="ExternalInput").ap()
    out = nc.dram_tensor(
        "out", [2, BPC, 2 * H, 2 * W, C], F32, kind="ExternalOutput"
    ).ap()
    xs = (xr, xi)
    EXPMAX = 2 * HF * C  # largest expanded chunk (8192 f32 = 32 KB/partition)

    # walrus codegen allows exactly ONE sync-wait command per engine
    # instruction (multi-wait is only legal on Drain/EventSemaphore). Tile
    # emits a wait only when the issuing engine has not already observed
    # that semaphore tick through an earlier *real* instruction's wait
    # (InstWrite/NoOps don't count). Every instruction below is budgeted to
    # observe at most one fresh tick, using tiny absorber instructions
    # (1-element memsets on DVE, 2-element probe copies on ACT) to
    # pre-observe everything else; a DMA's remaining single wait is then
    # its own-lane predecessor completion.
    with TileContext(nc) as tc:
        with (
            tc.tile_pool(name="pin", bufs=LD_BUFS) as pin,
            tc.tile_pool(name="pinit", bufs=LD_BUFS) as pinit,
            tc.tile_pool(name="pout", bufs=CP_BUFS) as pout,
            tc.tile_pool(name="pdummy", bufs=1) as pdummy,
        ):
            dummy = pdummy.tile([H, 2 * N_ITERS], F32, name="dummy")
            vdummy = pdummy.tile([H, 4 * N_ITERS], F32, name="vdummy")
            spdummy = pdummy.tile([1, 16], F32, name="spdummy")

            # Pre-create the tin tiles so chunk (k-3)'s ACT stream can issue
            # ld(k)'s DMA.
            tins = []
            for k, (t, b, w0, wlen) in enumerate(_FLAT):
                if k < LD_BUFS:
                    # dedicated, never-recycled tiles: the ramp loads carry
                    # no WAR at all
                    tins.append(pinit.tile([H, wlen * C], F32, name="tin_init"))
                else:
                    tins.append(None)  # allocated lazily inside the loop

            lds = [None] * N_ITERS
            cps = []
            st_los = []
            st_his = []
            aabs_all = []
            # Ramp loads: SP HWDGE lanes 0/1 (fast first byte, two lanes so
            # they overlap), issued before everything else.
            for k in range(LD_BUFS):
                t, b, w0, wlen = _FLAT[k]
                lds[k] = nc.sync.dma_start(
                    out=tins[k][:, : wlen * C],
                    in_=xs[t][b, :, w0 : w0 + wlen, :],
                )

            for k, (t, b, w0, wlen) in enumerate(_FLAT):
                tin = tins[k]

                # ---- expand (one copy; both output rows read it) ----
                # DVE absorbers: per-iter distinct scratch cells. vabs1/2
                # observe the (k-CP_BUFS) store pair's lane ticks (tout
                # slot WAR); vabs3 observes the newest ACT probe (probe
                # WAR on the recycled tout slot); vabs4 observes cp(k-1)'s
                # own-sem tick (the recycled slots' release bundles land
                # there on the DVE timeline).
                vabs1 = nc.vector.memset(vdummy[:1, 4 * k : 4 * k + 1], 0.0)
                vabs2 = nc.vector.memset(vdummy[:1, 4 * k + 1 : 4 * k + 2], 0.0)
                vabs3 = nc.vector.memset(vdummy[:1, 4 * k + 2 : 4 * k + 3], 0.0)
                vabs4 = nc.vector.memset(vdummy[:1, 4 * k + 3 : 4 * k + 4], 0.0)
                if k >= CP_BUFS:
                    add_dep_helper(
                        vabs1.ins, st_los[k - CP_BUFS].ins, sync=True,
                        reason="absorb tout slot WAR (store-lo lane)",
                    )
                    add_dep_helper(
                        vabs2.ins, st_his[k - CP_BUFS].ins, sync=True,
                        reason="absorb tout slot WAR (store-hi lane)",
                    )
                if k >= 1:
                    add_dep_helper(
                        vabs3.ins, aabs_all[k - 1].ins, sync=True,
                        reason="absorb probe WAR (ACT sem)",
                    )
                    add_dep_helper(
                        vabs4.ins, cps[k - 1].ins, sync=True,
                        reason="absorb slot releases (DVE self sem)",
                    )
                tout = pout.tile([H, EXPMAX], F32, name="tout")
                EXP = 2 * wlen * C
                src = (
                    tin[:, : wlen * C]
                    .rearrange("p (w c) -> p w c", c=C)
                    .unsqueeze(2)
                    .broadcast_to([H, wlen, 2, C])
                )
                dst = tout[:, :EXP].rearrange("p (w s c) -> p w s c", s=2, c=C)
                cp = nc.vector.tensor_copy(out=dst, in_=src)
                for vb in (vabs1, vabs2, vabs3, vabs4):
                    add_dep_helper(
                        cp.ins, vb.ins, sync=False,
                        reason="absorbers run before copy",
                    )
                cps.append(cp)

                # ---- stores (both rows from the same expanded tile) ----
                # One 2-element ACT probe absorbs the DVE data tick; both
                # stores then carry only their own-lane predecessor wait
                # (~2 chunks old -> satisfied on arrival).
                ov = out[t, b].rearrange("(i r) w c -> i r (w c)", r=2)
                o0 = 2 * w0 * C
                aabs = nc.scalar.copy(
                    out=dummy[:1, 2 * k : 2 * k + 2], in_=tout[:1, 0:2]
                )
                aabs_all.append(aabs)
                st_lo = nc.scalar.dma_start(
                    out=ov[:, 0, o0 : o0 + EXP], in_=tout[:, :EXP]
                )
                add_dep_helper(
                    st_lo.ins, aabs.ins, sync=False,
                    reason="probe runs before store",
                )
                st_hi = nc.scalar.dma_start(
                    out=ov[:, 1, o0 : o0 + EXP], in_=tout[:, :EXP]
                )
                add_dep_helper(
                    st_hi.ins, st_lo.ins, sync=False,
                    reason="pair stores issue back to back",
                )
                st_los.append(st_lo)
                st_his.append(st_hi)

                # ---- prefetch load for chunk k+LD_BUFS-1 on the ACT ring
                # (enqueued 3 chunks early; the probe above has already
                # observed cp(k), which covers the release bundle of the
                # tin slot this load recycles).
                j = k + LD_BUFS - 1
                if LD_BUFS <= j < N_ITERS:
                    tj, bj, w0j, wlenj = _FLAT[j]
                    tins[j] = pin.tile([H, HF * C], F32, name="tin")
                    ld = nc.scalar.dma_start(
                        out=tins[j][:, : wlenj * C],
                        in_=xs[tj][bj, :, w0j : w0j + wlenj, :],
                    )
                    add_dep_helper(
                        ld.ins, st_hi.ins, sync=False,
                        reason="load rides the store ring after the pair",
                    )
                    lds[j] = ld

            # Kernel-tail absorbers: Tile's final SP drain waits on every
            # outstanding proc, but a multi-wait drain lowers to a 1-wait
            # NOP struct when cheap. Pre-observe each proc with one 4-byte
            # SP write per tick: the newest DMA on each of the 6 ACT lanes
            # (the last 3 store pairs -- the final loads are older), the
            # SP-lane ramp loads, the last copy (DVE) and the last probe
            # (ACT).
            tail_deps = []
            for j in range(3):
                tail_deps += [st_los[N_ITERS - 1 - j], st_his[N_ITERS - 1 - j]]
            tail_deps += [lds[2], lds[3], cps[-1], aabs_all[-1]]
            for j, dep in enumerate(tail_deps):
                wr = nc.sync.write(spdummy[:1, j : j + 1], b"\x00\x00\x00\x00")
                add_dep_helper(
                    wr.ins, dep.ins, sync=True,
                    reason="pre-observe outstanding procs for tail drain",
                )
    return nc


_NC_CACHE: bass.Bass | None = None


def _get_nc() -> bass.Bass:
    global _NC_CACHE
    if _NC_CACHE is None:
        _NC_CACHE = _build()
    return _NC_CACHE


def _run(x_real: np.ndarray, x_imag: np.ndarray, **spmd_kwargs):
    x_real = np.ascontiguousarray(np.asarray(x_real, dtype=np.float32))
    x_imag = np.ascontiguousarray(np.asarray(x_imag, dtype=np.float32))
    assert x_real.shape == (B, H, W, C), x_real.shape
    assert x_imag.shape == (B, H, W, C), x_imag.shape
    in_maps = [
        {
            "x_real": x_real[c * BPC : (c + 1) * BPC],
            "x_imag": x_imag[c * BPC : (c + 1) * BPC],
        }
        for c in range(N_CORES)
    ]
    res = run_bass_kernel_spmd(
        _get_nc(), in_maps, core_ids=list(range(N_CORES)), **spmd_kwargs
    )
    full = np.concatenate([r["out"] for r in res.results], axis=1)
    return full, res


def kernel(x_real: np.ndarray, x_imag: np.ndarray) -> np.ndarray:
    full, _ = _run(x_real, x_imag)
    return full
